# revision 34
# baseline (speedup 1.0000x reference)
"""Trainium2 Bass kernel for nn_DownwardPropagation.

Math (per batch row b, channel c, layers l=1..L):
    fd_l = fd_{l-1} * td_l                       (direct downward flux)
    ff_l = (tdf_l + tmf_l) * ff_{l-1} + fd_{l-1} * tmd_l
    up_l = fd_{l-1} * rbd_l + ff_{l-1} * rbf_l
    ab_l = fd_{l-1} * atd_l + ff_{l-1} * atf_l
Outputs are channel sums: fd_sums[B, L+1], ff_sums[B, L+1],
up_sums[B, L+1] (up_0 = sum_c fd_0 * r_multi_direct), ab_sums[B, L].

Kernel layout: batch sharded 8 ways (2048 rows/core), each core processes
16 chunks of 128 rows (partition dim). Per chunk, layers are processed in
blocks of LB. The layer recurrences run as ONE tensor_tensor_scan per
carry per block over a c-major "grid" layout [C, G=LB+1]: slot (c,0) is a
boundary element with data0=0 (kills the cross-channel state leak) and
data1=carry-in (injects the per-channel initial value); slots (c,1..LB)
hold the per-layer coefficients. The scan output grid then holds
fd_{l0..l0+LB} per channel directly.
"""

import numpy as np

_B, _L, _C = 16384, 60, 48
_NCORES = 8
_P = 128
_LB = 20

PROPS = [
    "t_direct", "t_diffuse", "t_multi_direct", "t_multi_diffuse",
    "r_bottom_multi_direct", "r_bottom_multi_diffuse",
    "a_top_multi_direct", "a_top_multi_diffuse",
]
FLUX = ["r_multi_direct", "flux_down_above_direct", "flux_down_above_diffuse"]


def build_nc(n_rows=_B // _NCORES, L=_L, C=_C, LB=_LB, n_cores=_NCORES):
    import concourse.bacc as bacc
    import concourse.mybir as mybir
    from concourse.tile import TileContext

    f32 = mybir.dt.float32
    AL = mybir.AluOpType
    AX = mybir.AxisListType
    P = _P
    assert n_rows % P == 0 and L % LB == 0
    n_chunks = n_rows // P
    NBLK = L // LB
    G = LB + 1

    nc = bacc.Bacc("TRN2", target_bir_lowering=False, debug=False,
                   num_devices=n_cores)

    d_in = {n: nc.dram_tensor(n, [n_rows, L, C], f32, kind="ExternalInput").ap()
            for n in PROPS}
    d_fx = {n: nc.dram_tensor(n, [n_rows, C], f32, kind="ExternalInput").ap()
            for n in FLUX}
    d_fds = nc.dram_tensor("out_fds", [n_rows, L + 1], f32, kind="ExternalOutput").ap()
    d_ffs = nc.dram_tensor("out_ffs", [n_rows, L + 1], f32, kind="ExternalOutput").ap()
    d_ups = nc.dram_tensor("out_ups", [n_rows, L + 1], f32, kind="ExternalOutput").ap()
    d_abs = nc.dram_tensor("out_abs", [n_rows, L], f32, kind="ExternalOutput").ap()

    with TileContext(nc) as tc:
        with (
            tc.tile_pool(name="inp", bufs=2) as pool_in,
            tc.tile_pool(name="grids", bufs=2) as pool_grid,
            tc.tile_pool(name="seq", bufs=3) as pool_seq,
            tc.tile_pool(name="prod", bufs=3) as pool_prod,
            tc.tile_pool(name="outs", bufs=4) as pool_out,
            tc.tile_pool(name="small", bufs=4) as pool_small,
            tc.tile_pool(name="persist", bufs=1) as pool_persist,
        ):
            # Persistent injection grids, alternating per chunk parity.
            # Zeroed once; only their boundary column (tdc/ac: never) is
            # rewritten afterwards, so the zero interior persists.
            fd1 = [pool_persist.tile([P, C * G], f32, name=f"fd1_{i}", tag=f"fd1_{i}") for i in range(2)]
            tdc = [pool_persist.tile([P, C * G], f32, name=f"tdc_{i}", tag=f"tdc_{i}") for i in range(2)]
            ac = [pool_persist.tile([P, C * G], f32, name=f"ac_{i}", tag=f"ac_{i}") for i in range(2)]
            for t in (*fd1, *tdc, *ac):
                nc.vector.memset(t[:], 0.0)

            def co3(t):  # c-order view of an l-major [P, LB*C] tile -> [P, C, LB]
                return t.rearrange("p (l c) -> p l c", c=C).transpose([0, 2, 1])

            def lm3(t):  # l-major view [P, LB, C], contiguous
                return t.rearrange("p (l c) -> p l c", c=C)

            def chunk_prologue(ch):
                r0 = ch * P
                st = {"ch": ch, "r0": r0}
                st["fd1_3"] = fd1[ch % 2].rearrange("p (c g) -> p c g", g=G)
                st["tdc_g"], st["ac_g"], st["fd1_g"] = \
                    tdc[ch % 2], ac[ch % 2], fd1[ch % 2]
                st["tdc_3"] = tdc[ch % 2].rearrange("p (c g) -> p c g", g=G)
                st["ac_3"] = ac[ch % 2].rearrange("p (c g) -> p c g", g=G)

                fd0 = pool_small.tile([P, C], f32, tag="fd0")
                ff0 = pool_small.tile([P, C], f32, tag="ff0")
                rmd = pool_small.tile([P, C], f32, tag="rmd")
                nc.sync.dma_start(out=fd0[:], in_=d_fx["flux_down_above_direct"][r0:r0 + P])
                nc.sync.dma_start(out=ff0[:], in_=d_fx["flux_down_above_diffuse"][r0:r0 + P])
                nc.sync.dma_start(out=rmd[:], in_=d_fx["r_multi_direct"][r0:r0 + P])
                st["fd0"], st["ff0"] = fd0, ff0

                # packed output tiles: [fds | ffs] and [ups | abs'] in
                # 64-col halves so each block needs only ONE reduce per tile
                # (abs is stored shifted +1 col; the epilogue re-slices)
                st["t_fdffs"] = pool_out.tile([P, 128], f32, name="t_fdffs",
                                              tag="o_fdffs")
                st["t_upabs"] = pool_out.tile([P, 128], f32, name="t_upabs",
                                              tag="o_upabs")

                # up_0 = sum_c fd0 * rmd  (accum_out of a fused mult)
                trash = pool_small.tile([P, C], f32, tag="trash")
                nc.vector.scalar_tensor_tensor(
                    out=trash[:], in0=fd0[:], scalar=1.0, in1=rmd[:],
                    op0=AL.mult, op1=AL.mult, accum_out=st["t_upabs"][:, 0:1])
                # level-0 channel sums of the initial fluxes
                nc.vector.tensor_reduce(out=st["t_fdffs"][:, 0:1], in_=fd0[:],
                                        axis=AX.X, op=AL.add)
                nc.vector.tensor_reduce(out=st["t_fdffs"][:, 64:65], in_=ff0[:],
                                        axis=AX.X, op=AL.add)
                return st

            lane3 = lambda t: t.rearrange("p (l c) -> p l c", c=C)

            def block_h1(st, bl):
                """First half of a block: loads, grid staging, fd scan,
                b-mult, fd-based products."""
                r0, l0 = st["r0"], bl * LB
                fd1_3, tdc_3, ac_3 = st["fd1_3"], st["tdc_3"], st["ac_3"]
                tin = {}
                for name in ("t_direct", "t_multi_direct"):
                    t = pool_in.tile([P, LB * C], f32, name=f"in_{name}",
                                     tag=f"in_{name}", bufs=3)
                    # second HWDGE ring (qActDynamicHW) — spreads DMA
                    # descriptor generation across both physical rings
                    nc.scalar.dma_start(
                        out=t[:],
                        in_=d_in[name][r0:r0 + P, l0:l0 + LB].rearrange(
                            "p l c -> p (l c)"))
                    tin[name] = t
                # rbd|rbf and atd|atf pair-packed so each product pair is a
                # single DVE op (halves per-op fixed cost and DRAIN count)
                rr = pool_in.tile([P, 2 * LB * C], f32, name="in_rr",
                                  tag="in_rr", bufs=2)
                aa = pool_in.tile([P, 2 * LB * C], f32, name="in_aa",
                                  tag="in_aa", bufs=2)
                for dst, name in ((rr[:, 0:LB * C], "r_bottom_multi_direct"),
                                  (rr[:, LB * C:], "r_bottom_multi_diffuse"),
                                  (aa[:, 0:LB * C], "a_top_multi_direct"),
                                  (aa[:, LB * C:], "a_top_multi_diffuse")):
                    nc.sync.dma_start(
                        out=dst,
                        in_=d_in[name][r0:r0 + P, l0:l0 + LB].rearrange(
                            "p l c -> p (l c)"))

                # a = tdf + tmf computed by the DMA engines: load tdf, then
                # accumulate tmf onto it in the SDMA CCE (SWDGE path; the Q7
                # descriptor-generation cost is free now that GPSIMD idles)
                a_lm = pool_grid.tile([P, LB * C], f32, tag="a_lm", bufs=3)
                nc.scalar.dma_start(
                    out=a_lm[:],
                    in_=d_in["t_diffuse"][r0:r0 + P, l0:l0 + LB].rearrange(
                        "p l c -> p (l c)"))
                nc.gpsimd.dma_start(
                    out=a_lm[:],
                    in_=d_in["t_multi_diffuse"][r0:r0 + P, l0:l0 + LB].rearrange(
                        "p l c -> p (l c)"),
                    accum_op=AL.add)

                # Input-only ACT copies (depend only on DMAs)
                nc.scalar.copy(out=tdc_3[:, :, 1:G], in_=co3(tin["t_direct"]))
                tmd_c = pool_grid.tile([P, C * G], f32, tag="tmd_c")
                tmdc_3 = tmd_c.rearrange("p (c g) -> p c g", g=G)
                nc.scalar.copy(out=tmdc_3[:, :, 1:G],
                               in_=co3(tin["t_multi_direct"]))
                nc.scalar.copy(out=ac_3[:, :, 1:G], in_=co3(a_lm))

                # carry-critical boundary copy on the (idle) GPSIMD
                if bl == 0:
                    nc.gpsimd.tensor_copy(out=fd1_3[:, :, 0:1],
                                          in_=st["fd0"][:].unsqueeze(2))
                else:
                    nc.gpsimd.tensor_copy(out=fd1_3[:, :, 0:1],
                                          in_=st["prev_fd_c3"][:, :, LB:LB + 1])

                fd_c = pool_seq.tile([P, C * G], f32, tag="fd_c")
                nc.vector.tensor_tensor_scan(
                    out=fd_c[:], data0=st["tdc_g"][:], data1=st["fd1_g"][:],
                    initial=0.0, op0=AL.mult, op1=AL.add)
                fd_c3 = fd_c.rearrange("p (c g) -> p c g", g=G)

                # b grid: col0 = ff carry, cols 1.. = fd_{l-1} * tmd_l
                b_buf = pool_grid.tile([P, C * G], f32, tag="b_buf", bufs=2)
                b_3 = b_buf.rearrange("p (c g) -> p c g", g=G)
                if bl == 0:
                    nc.gpsimd.tensor_copy(out=b_3[:, :, 0:1],
                                          in_=st["ff0"][:].unsqueeze(2))
                else:
                    nc.gpsimd.tensor_copy(out=b_3[:, :, 0:1],
                                          in_=st["prev_ff_c3"][:, :, LB:LB + 1])
                nc.gpsimd.tensor_mul(out=b_3[:, :, 1:G],
                                     in0=fd_c3[:, :, 0:LB],
                                     in1=tmdc_3[:, :, 1:G])
                lanes = pool_prod.tile([P, 2 * G * C], f32, tag="lanes", bufs=2)
                fd_lane = lanes[:, 0:G * C]

                quad = pool_prod.tile([P, 4 * LB * C], f32, tag="quad", bufs=2)
                return {"bl": bl, "tin": tin, "fd_c3": fd_c3, "b_buf": b_buf,
                        "b_3": b_3, "lanes": lanes, "fd_lane": fd_lane,
                        "rr": rr, "aa": aa, "quad": quad}

            def block_h2(st, bx):
                """Second half: ff scan and ff-based products."""
                bl = bx["bl"]
                ff_c = pool_seq.tile([P, C * G], f32, tag="ff_c")
                nc.vector.tensor_tensor_scan(
                    out=ff_c[:], data0=st["ac_g"][:], data1=bx["b_buf"][:],
                    initial=0.0, op0=AL.mult, op1=AL.add)
                ff_c3 = ff_c.rearrange("p (c g) -> p c g", g=G)

                nc.scalar.copy(out=lane3(bx["fd_lane"]),
                               in_=bx["fd_c3"].transpose([0, 2, 1]))
                ff_lane = bx["lanes"][:, G * C:]
                nc.scalar.copy(out=lane3(ff_lane),
                               in_=ff_c3.transpose([0, 2, 1]))

                lanes_u = bx["lanes"].rearrange("p (t x) -> p t x", t=2
                                                )[:, :, 0:LB * C]
                nc.vector.tensor_mul(
                    out=bx["quad"][:, 0:2 * LB * C].rearrange(
                        "p (t x) -> p t x", t=2),
                    in0=lanes_u,
                    in1=bx["rr"].rearrange("p (t x) -> p t x", t=2))
                nc.vector.tensor_mul(
                    out=bx["quad"][:, 2 * LB * C:].rearrange(
                        "p (t x) -> p t x", t=2),
                    in0=lanes_u,
                    in1=bx["aa"].rearrange("p (t x) -> p t x", t=2))

                bx["ff_lane"] = ff_lane
                st["prev_fd_c3"], st["prev_ff_c3"] = bx["fd_c3"], ff_c3

            def block_tail(st, bx):
                """Block tail: the four DVE reductions (optionally emitted
                SKEW steps later in the stream)."""
                l0 = bx["bl"] * LB
                # one XY-reduce for up+ab: [P, pair, l, t, c], sum (t, c)
                q_red = bx["quad"].rearrange(
                    "p (r t l c) -> p r t l c", r=2, t=2, c=C
                ).transpose([0, 1, 3, 2, 4])
                nc.vector.tensor_reduce(
                    out=st["t_upabs"].rearrange("p (r x) -> p r x", r=2
                                                )[:, :, l0 + 1:l0 + LB + 1],
                    in_=q_red, axis=AX.XY, op=AL.add)
                # one X-reduce for both level-sum lanes (fd | ff)
                l_red = bx["lanes"].rearrange(
                    "p (t g c) -> p t g c", t=2, c=C)[:, :, 1:G, :]
                nc.vector.tensor_reduce(
                    out=st["t_fdffs"].rearrange("p (t x) -> p t x", t=2
                                                )[:, :, l0 + 1:l0 + LB + 1],
                    in_=l_red, axis=AX.X, op=AL.add)

            def chunk_epilogue(st):
                r0 = st["r0"]
                nc.sync.dma_start(out=d_fds[r0:r0 + P],
                                  in_=st["t_fdffs"][:, 0:L + 1])
                nc.sync.dma_start(out=d_ffs[r0:r0 + P],
                                  in_=st["t_fdffs"][:, 64:64 + L + 1])
                nc.sync.dma_start(out=d_ups[r0:r0 + P],
                                  in_=st["t_upabs"][:, 0:L + 1])
                nc.sync.dma_start(out=d_abs[r0:r0 + P],
                                  in_=st["t_upabs"][:, 65:65 + L])

            # Software-pipeline two independent chunks at block granularity
            # in one continuous stream: each engine's FIFO alternates
            # between two carry chains (no head-of-line stalls), and
            # prologues are emitted a few steps early so the next pair's
            # flux DMAs / output tiles are ready before its first block.
            seq = []
            for p in range(0, n_chunks, 2):
                for bl in range(NBLK):
                    for s_ in range(min(2, n_chunks - p)):
                        seq.append((p + s_, bl))
            states = {}

            def ensure_prologue(ch):
                if ch not in states:
                    states[ch] = chunk_prologue(ch)

            SKEW = 0
            pending = []  # (state, block-context) awaiting the skewed tail
            for idx, (ch, bl) in enumerate(seq):
                ensure_prologue(ch)
                for ch2, _ in seq[idx + 1:idx + 5]:
                    ensure_prologue(ch2)
                bx = block_h1(states[ch], bl)
                block_h2(states[ch], bx)
                pending.append((states[ch], bx))
                if len(pending) > SKEW:
                    st2, bx2 = pending.pop(0)
                    block_tail(st2, bx2)
                    if bx2["bl"] == NBLK - 1:
                        chunk_epilogue(st2)
            for st2, bx2 in pending:
                block_tail(st2, bx2)
                if bx2["bl"] == NBLK - 1:
                    chunk_epilogue(st2)

    nc.compile()
    return nc


_NC_CACHE = {}


def _get_nc(key=("full",)):
    if key not in _NC_CACHE:
        _NC_CACHE[key] = build_nc()
    return _NC_CACHE[key]


def kernel(**inputs):
    """Full-problem entry point: shard over 8 cores, run, gather."""
    from concourse.bass_utils import run_bass_kernel_spmd

    nc = _get_nc()
    rows = _B // _NCORES
    in_maps = []
    for core in range(_NCORES):
        sl = slice(core * rows, (core + 1) * rows)
        m = {n: np.ascontiguousarray(np.asarray(inputs[n])[sl], dtype=np.float32)
             for n in PROPS + FLUX}
        in_maps.append(m)

    res = run_bass_kernel_spmd(nc, in_maps, core_ids=list(range(_NCORES)))
    fds = np.concatenate([r["out_fds"] for r in res.results], axis=0)
    ffs = np.concatenate([r["out_ffs"] for r in res.results], axis=0)
    ups = np.concatenate([r["out_ups"] for r in res.results], axis=0)
    abs_ = np.concatenate([r["out_abs"] for r in res.results], axis=0)
    return fds, ffs, ups, abs_


# revision 36
# speedup vs baseline: 1.0164x; 1.0164x over previous
"""Trainium2 Bass kernel for nn_DownwardPropagation.

Math (per batch row b, channel c, layers l=1..L):
    fd_l = fd_{l-1} * td_l                       (direct downward flux)
    ff_l = (tdf_l + tmf_l) * ff_{l-1} + fd_{l-1} * tmd_l
    up_l = fd_{l-1} * rbd_l + ff_{l-1} * rbf_l
    ab_l = fd_{l-1} * atd_l + ff_{l-1} * atf_l
Outputs are channel sums: fd_sums[B, L+1], ff_sums[B, L+1],
up_sums[B, L+1] (up_0 = sum_c fd_0 * r_multi_direct), ab_sums[B, L].

Kernel layout: batch sharded 8 ways (2048 rows/core), each core processes
16 chunks of 128 rows (partition dim). Per chunk, layers are processed in
blocks of LB. The layer recurrences run as ONE tensor_tensor_scan per
carry per block over a c-major "grid" layout [C, G=LB+1]: slot (c,0) is a
boundary element with data0=0 (kills the cross-channel state leak) and
data1=carry-in (injects the per-channel initial value); slots (c,1..LB)
hold the per-layer coefficients. The scan output grid then holds
fd_{l0..l0+LB} per channel directly.
"""

import numpy as np

_B, _L, _C = 16384, 60, 48
_NCORES = 8
_P = 128
_LB = 20

PROPS = [
    "t_direct", "t_diffuse", "t_multi_direct", "t_multi_diffuse",
    "r_bottom_multi_direct", "r_bottom_multi_diffuse",
    "a_top_multi_direct", "a_top_multi_diffuse",
]
FLUX = ["r_multi_direct", "flux_down_above_direct", "flux_down_above_diffuse"]


def build_nc(n_rows=_B // _NCORES, L=_L, C=_C, LB=_LB, n_cores=_NCORES):
    import concourse.bacc as bacc
    import concourse.mybir as mybir
    from concourse.tile import TileContext

    f32 = mybir.dt.float32
    AL = mybir.AluOpType
    AX = mybir.AxisListType
    P = _P
    assert n_rows % P == 0 and L % LB == 0
    n_chunks = n_rows // P
    NBLK = L // LB
    G = LB + 1

    nc = bacc.Bacc("TRN2", target_bir_lowering=False, debug=False,
                   num_devices=n_cores)

    d_in = {n: nc.dram_tensor(n, [n_rows, L, C], f32, kind="ExternalInput").ap()
            for n in PROPS}
    d_fx = {n: nc.dram_tensor(n, [n_rows, C], f32, kind="ExternalInput").ap()
            for n in FLUX}
    d_fds = nc.dram_tensor("out_fds", [n_rows, L + 1], f32, kind="ExternalOutput").ap()
    d_ffs = nc.dram_tensor("out_ffs", [n_rows, L + 1], f32, kind="ExternalOutput").ap()
    d_ups = nc.dram_tensor("out_ups", [n_rows, L + 1], f32, kind="ExternalOutput").ap()
    d_abs = nc.dram_tensor("out_abs", [n_rows, L], f32, kind="ExternalOutput").ap()

    with TileContext(nc) as tc:
        with (
            tc.tile_pool(name="inp", bufs=2) as pool_in,
            tc.tile_pool(name="grids", bufs=2) as pool_grid,
            tc.tile_pool(name="seq", bufs=3) as pool_seq,
            tc.tile_pool(name="prod", bufs=3) as pool_prod,
            tc.tile_pool(name="outs", bufs=4) as pool_out,
            tc.tile_pool(name="small", bufs=4) as pool_small,
            tc.tile_pool(name="persist", bufs=1) as pool_persist,
        ):
            # Persistent injection grids, alternating per chunk parity.
            # Zeroed once; only their boundary column (tdc/ac: never) is
            # rewritten afterwards, so the zero interior persists.
            fd1 = [pool_persist.tile([P, C * G], f32, name=f"fd1_{i}", tag=f"fd1_{i}") for i in range(2)]
            tdc = [pool_persist.tile([P, C * G], f32, name=f"tdc_{i}", tag=f"tdc_{i}") for i in range(2)]
            ac = [pool_persist.tile([P, C * G], f32, name=f"ac_{i}", tag=f"ac_{i}") for i in range(2)]
            for t in (*fd1, *tdc, *ac):
                nc.vector.memset(t[:], 0.0)

            def co3(t):  # c-order view of an l-major [P, LB*C] tile -> [P, C, LB]
                return t.rearrange("p (l c) -> p l c", c=C).transpose([0, 2, 1])

            def lm3(t):  # l-major view [P, LB, C], contiguous
                return t.rearrange("p (l c) -> p l c", c=C)

            def chunk_prologue(ch):
                r0 = ch * P
                st = {"ch": ch, "r0": r0}
                st["fd1_3"] = fd1[ch % 2].rearrange("p (c g) -> p c g", g=G)
                st["tdc_g"], st["ac_g"], st["fd1_g"] = \
                    tdc[ch % 2], ac[ch % 2], fd1[ch % 2]
                st["tdc_3"] = tdc[ch % 2].rearrange("p (c g) -> p c g", g=G)
                st["ac_3"] = ac[ch % 2].rearrange("p (c g) -> p c g", g=G)

                fd0 = pool_small.tile([P, C], f32, tag="fd0")
                ff0 = pool_small.tile([P, C], f32, tag="ff0")
                rmd = pool_small.tile([P, C], f32, tag="rmd")
                nc.sync.dma_start(out=fd0[:], in_=d_fx["flux_down_above_direct"][r0:r0 + P])
                nc.sync.dma_start(out=ff0[:], in_=d_fx["flux_down_above_diffuse"][r0:r0 + P])
                nc.sync.dma_start(out=rmd[:], in_=d_fx["r_multi_direct"][r0:r0 + P])
                st["fd0"], st["ff0"] = fd0, ff0

                # packed output tiles: [fds | ffs] and [ups | abs'] in
                # 64-col halves so each block needs only ONE reduce per tile
                # (abs is stored shifted +1 col; the epilogue re-slices)
                st["t_fdffs"] = pool_out.tile([P, 128], f32, name="t_fdffs",
                                              tag="o_fdffs")
                st["t_upabs"] = pool_out.tile([P, 128], f32, name="t_upabs",
                                              tag="o_upabs")

                # up_0 = sum_c fd0 * rmd  (accum_out of a fused mult)
                trash = pool_small.tile([P, C], f32, tag="trash")
                nc.vector.scalar_tensor_tensor(
                    out=trash[:], in0=fd0[:], scalar=1.0, in1=rmd[:],
                    op0=AL.mult, op1=AL.mult, accum_out=st["t_upabs"][:, 0:1])
                # level-0 channel sums of the initial fluxes
                nc.vector.tensor_reduce(out=st["t_fdffs"][:, 0:1], in_=fd0[:],
                                        axis=AX.X, op=AL.add)
                nc.vector.tensor_reduce(out=st["t_fdffs"][:, 64:65], in_=ff0[:],
                                        axis=AX.X, op=AL.add)
                return st

            lane3 = lambda t: t.rearrange("p (l c) -> p l c", c=C)

            def block_h1(st, bl):
                """First half of a block: loads, grid staging, fd scan,
                b-mult, fd-based products."""
                r0, l0 = st["r0"], bl * LB
                fd1_3, tdc_3, ac_3 = st["fd1_3"], st["tdc_3"], st["ac_3"]
                tin = {}
                for name in ("t_direct", "t_multi_direct"):
                    t = pool_in.tile([P, LB * C], f32, name=f"in_{name}",
                                     tag=f"in_{name}", bufs=3)
                    nc.sync.dma_start(
                        out=t[:],
                        in_=d_in[name][r0:r0 + P, l0:l0 + LB].rearrange(
                            "p l c -> p (l c)"))
                    tin[name] = t
                # all four product coefficients packed in one tile so the
                # whole product stage is a single DVE op
                coef = pool_in.tile([P, 4 * LB * C], f32, name="in_coef",
                                    tag="in_coef", bufs=2)
                for q, name in enumerate(("r_bottom_multi_direct",
                                          "r_bottom_multi_diffuse",
                                          "a_top_multi_direct",
                                          "a_top_multi_diffuse")):
                    nc.sync.dma_start(
                        out=coef[:, q * LB * C:(q + 1) * LB * C],
                        in_=d_in[name][r0:r0 + P, l0:l0 + LB].rearrange(
                            "p l c -> p (l c)"))

                # a = tdf + tmf computed by the DMA engines: load tdf, then
                # accumulate tmf onto it in the SDMA CCE (SWDGE path; the Q7
                # descriptor-generation cost is free now that GPSIMD idles)
                a_lm = pool_grid.tile([P, LB * C], f32, tag="a_lm", bufs=3)
                nc.sync.dma_start(
                    out=a_lm[:],
                    in_=d_in["t_diffuse"][r0:r0 + P, l0:l0 + LB].rearrange(
                        "p l c -> p (l c)"))
                nc.gpsimd.dma_start(
                    out=a_lm[:],
                    in_=d_in["t_multi_diffuse"][r0:r0 + P, l0:l0 + LB].rearrange(
                        "p l c -> p (l c)"),
                    accum_op=AL.add)

                # Input-only ACT copies (depend only on DMAs)
                nc.scalar.copy(out=tdc_3[:, :, 1:G], in_=co3(tin["t_direct"]))
                tmd_c = pool_grid.tile([P, C * G], f32, tag="tmd_c")
                tmdc_3 = tmd_c.rearrange("p (c g) -> p c g", g=G)
                nc.scalar.copy(out=tmdc_3[:, :, 1:G],
                               in_=co3(tin["t_multi_direct"]))
                nc.scalar.copy(out=ac_3[:, :, 1:G], in_=co3(a_lm))

                # carry-critical boundary copy on the (idle) GPSIMD
                if bl == 0:
                    nc.gpsimd.tensor_copy(out=fd1_3[:, :, 0:1],
                                          in_=st["fd0"][:].unsqueeze(2))
                else:
                    nc.gpsimd.tensor_copy(out=fd1_3[:, :, 0:1],
                                          in_=st["prev_fd_c3"][:, :, LB:LB + 1])

                fd_c = pool_seq.tile([P, C * G], f32, tag="fd_c")
                nc.vector.tensor_tensor_scan(
                    out=fd_c[:], data0=st["tdc_g"][:], data1=st["fd1_g"][:],
                    initial=0.0, op0=AL.mult, op1=AL.add)
                fd_c3 = fd_c.rearrange("p (c g) -> p c g", g=G)

                # b grid: col0 = ff carry, cols 1.. = fd_{l-1} * tmd_l
                b_buf = pool_grid.tile([P, C * G], f32, tag="b_buf", bufs=2)
                b_3 = b_buf.rearrange("p (c g) -> p c g", g=G)
                if bl == 0:
                    nc.gpsimd.tensor_copy(out=b_3[:, :, 0:1],
                                          in_=st["ff0"][:].unsqueeze(2))
                else:
                    nc.gpsimd.tensor_copy(out=b_3[:, :, 0:1],
                                          in_=st["prev_ff_c3"][:, :, LB:LB + 1])
                nc.gpsimd.tensor_mul(out=b_3[:, :, 1:G],
                                     in0=fd_c3[:, :, 0:LB],
                                     in1=tmdc_3[:, :, 1:G])
                lanes = pool_prod.tile([P, 2 * G * C], f32, tag="lanes", bufs=2)
                fd_lane = lanes[:, 0:G * C]

                quad = pool_prod.tile([P, 4 * LB * C], f32, tag="quad", bufs=2)
                return {"bl": bl, "tin": tin, "fd_c3": fd_c3, "b_buf": b_buf,
                        "b_3": b_3, "lanes": lanes, "fd_lane": fd_lane,
                        "coef": coef, "quad": quad}

            def block_h2(st, bx):
                """Second half: ff scan and ff-based products."""
                bl = bx["bl"]
                ff_c = pool_seq.tile([P, C * G], f32, tag="ff_c")
                nc.vector.tensor_tensor_scan(
                    out=ff_c[:], data0=st["ac_g"][:], data1=bx["b_buf"][:],
                    initial=0.0, op0=AL.mult, op1=AL.add)
                ff_c3 = ff_c.rearrange("p (c g) -> p c g", g=G)

                nc.scalar.copy(out=lane3(bx["fd_lane"]),
                               in_=bx["fd_c3"].transpose([0, 2, 1]))
                ff_lane = bx["lanes"][:, G * C:]
                nc.scalar.copy(out=lane3(ff_lane),
                               in_=ff_c3.transpose([0, 2, 1]))

                # single product op: lanes broadcast (step-0 dim) against
                # the packed coefficients -> quad = u0|u1|a0|a1
                lanes_b = bx["lanes"].rearrange("p (t x) -> p t x", t=2
                                                )[:, :, 0:LB * C].unsqueeze(1
                                                ).broadcast_to(
                                                    [P, 2, 2, LB * C])
                nc.vector.tensor_mul(
                    out=bx["quad"].rearrange("p (r t x) -> p r t x", r=2, t=2),
                    in0=lanes_b,
                    in1=bx["coef"].rearrange("p (r t x) -> p r t x", r=2, t=2))

                bx["ff_lane"] = ff_lane
                st["prev_fd_c3"], st["prev_ff_c3"] = bx["fd_c3"], ff_c3

            def block_tail(st, bx):
                """Block tail: the four DVE reductions (optionally emitted
                SKEW steps later in the stream)."""
                l0 = bx["bl"] * LB
                # one XY-reduce for up+ab: [P, pair, l, t, c], sum (t, c)
                q_red = bx["quad"].rearrange(
                    "p (r t l c) -> p r t l c", r=2, t=2, c=C
                ).transpose([0, 1, 3, 2, 4])
                nc.vector.tensor_reduce(
                    out=st["t_upabs"].rearrange("p (r x) -> p r x", r=2
                                                )[:, :, l0 + 1:l0 + LB + 1],
                    in_=q_red, axis=AX.XY, op=AL.add)
                # one X-reduce for both level-sum lanes (fd | ff)
                l_red = bx["lanes"].rearrange(
                    "p (t g c) -> p t g c", t=2, c=C)[:, :, 1:G, :]
                nc.vector.tensor_reduce(
                    out=st["t_fdffs"].rearrange("p (t x) -> p t x", t=2
                                                )[:, :, l0 + 1:l0 + LB + 1],
                    in_=l_red, axis=AX.X, op=AL.add)

            def chunk_epilogue(st):
                r0 = st["r0"]
                nc.sync.dma_start(out=d_fds[r0:r0 + P],
                                  in_=st["t_fdffs"][:, 0:L + 1])
                nc.sync.dma_start(out=d_ffs[r0:r0 + P],
                                  in_=st["t_fdffs"][:, 64:64 + L + 1])
                nc.sync.dma_start(out=d_ups[r0:r0 + P],
                                  in_=st["t_upabs"][:, 0:L + 1])
                nc.sync.dma_start(out=d_abs[r0:r0 + P],
                                  in_=st["t_upabs"][:, 65:65 + L])

            # Software-pipeline two independent chunks at block granularity
            # in one continuous stream: each engine's FIFO alternates
            # between two carry chains (no head-of-line stalls), and
            # prologues are emitted a few steps early so the next pair's
            # flux DMAs / output tiles are ready before its first block.
            seq = []
            for p in range(0, n_chunks, 2):
                for bl in range(NBLK):
                    for s_ in range(min(2, n_chunks - p)):
                        seq.append((p + s_, bl))
            states = {}

            def ensure_prologue(ch):
                if ch not in states:
                    states[ch] = chunk_prologue(ch)

            SKEW = 0
            pending = []  # (state, block-context) awaiting the skewed tail
            for idx, (ch, bl) in enumerate(seq):
                ensure_prologue(ch)
                for ch2, _ in seq[idx + 1:idx + 5]:
                    ensure_prologue(ch2)
                bx = block_h1(states[ch], bl)
                block_h2(states[ch], bx)
                pending.append((states[ch], bx))
                if len(pending) > SKEW:
                    st2, bx2 = pending.pop(0)
                    block_tail(st2, bx2)
                    if bx2["bl"] == NBLK - 1:
                        chunk_epilogue(st2)
            for st2, bx2 in pending:
                block_tail(st2, bx2)
                if bx2["bl"] == NBLK - 1:
                    chunk_epilogue(st2)

    nc.compile()
    return nc


_NC_CACHE = {}


def _get_nc(key=("full",)):
    if key not in _NC_CACHE:
        _NC_CACHE[key] = build_nc()
    return _NC_CACHE[key]


def kernel(**inputs):
    """Full-problem entry point: shard over 8 cores, run, gather."""
    from concourse.bass_utils import run_bass_kernel_spmd

    nc = _get_nc()
    rows = _B // _NCORES
    in_maps = []
    for core in range(_NCORES):
        sl = slice(core * rows, (core + 1) * rows)
        m = {n: np.ascontiguousarray(np.asarray(inputs[n])[sl], dtype=np.float32)
             for n in PROPS + FLUX}
        in_maps.append(m)

    res = run_bass_kernel_spmd(nc, in_maps, core_ids=list(range(_NCORES)))
    fds = np.concatenate([r["out_fds"] for r in res.results], axis=0)
    ffs = np.concatenate([r["out_ffs"] for r in res.results], axis=0)
    ups = np.concatenate([r["out_ups"] for r in res.results], axis=0)
    abs_ = np.concatenate([r["out_abs"] for r in res.results], axis=0)
    return fds, ffs, ups, abs_


# revision 37
# speedup vs baseline: 1.0233x; 1.0068x over previous
"""Trainium2 Bass kernel for nn_DownwardPropagation.

Math (per batch row b, channel c, layers l=1..L):
    fd_l = fd_{l-1} * td_l                       (direct downward flux)
    ff_l = (tdf_l + tmf_l) * ff_{l-1} + fd_{l-1} * tmd_l
    up_l = fd_{l-1} * rbd_l + ff_{l-1} * rbf_l
    ab_l = fd_{l-1} * atd_l + ff_{l-1} * atf_l
Outputs are channel sums: fd_sums[B, L+1], ff_sums[B, L+1],
up_sums[B, L+1] (up_0 = sum_c fd_0 * r_multi_direct), ab_sums[B, L].

Kernel layout: batch sharded 8 ways (2048 rows/core), each core processes
16 chunks of 128 rows (partition dim). Per chunk, layers are processed in
blocks of LB. The layer recurrences run as ONE tensor_tensor_scan per
carry per block over a c-major "grid" layout [C, G=LB+1]: slot (c,0) is a
boundary element with data0=0 (kills the cross-channel state leak) and
data1=carry-in (injects the per-channel initial value); slots (c,1..LB)
hold the per-layer coefficients. The scan output grid then holds
fd_{l0..l0+LB} per channel directly.
"""

import numpy as np

_B, _L, _C = 16384, 60, 48
_NCORES = 8
_P = 128
_LB = 20

PROPS = [
    "t_direct", "t_diffuse", "t_multi_direct", "t_multi_diffuse",
    "r_bottom_multi_direct", "r_bottom_multi_diffuse",
    "a_top_multi_direct", "a_top_multi_diffuse",
]
FLUX = ["r_multi_direct", "flux_down_above_direct", "flux_down_above_diffuse"]


def build_nc(n_rows=_B // _NCORES, L=_L, C=_C, LB=_LB, n_cores=_NCORES):
    import concourse.bacc as bacc
    import concourse.mybir as mybir
    from concourse.tile import TileContext

    f32 = mybir.dt.float32
    AL = mybir.AluOpType
    AX = mybir.AxisListType
    P = _P
    assert n_rows % P == 0 and L % LB == 0
    n_chunks = n_rows // P
    NBLK = L // LB
    G = LB + 1

    nc = bacc.Bacc("TRN2", target_bir_lowering=False, debug=False,
                   num_devices=n_cores)

    d_in = {n: nc.dram_tensor(n, [n_rows, L, C], f32, kind="ExternalInput").ap()
            for n in PROPS}
    d_fx = {n: nc.dram_tensor(n, [n_rows, C], f32, kind="ExternalInput").ap()
            for n in FLUX}
    d_fds = nc.dram_tensor("out_fds", [n_rows, L + 1], f32, kind="ExternalOutput").ap()
    d_ffs = nc.dram_tensor("out_ffs", [n_rows, L + 1], f32, kind="ExternalOutput").ap()
    d_ups = nc.dram_tensor("out_ups", [n_rows, L + 1], f32, kind="ExternalOutput").ap()
    d_abs = nc.dram_tensor("out_abs", [n_rows, L], f32, kind="ExternalOutput").ap()

    with TileContext(nc) as tc:
        with (
            tc.tile_pool(name="inp", bufs=2) as pool_in,
            tc.tile_pool(name="grids", bufs=2) as pool_grid,
            tc.tile_pool(name="seq", bufs=3) as pool_seq,
            tc.tile_pool(name="prod", bufs=3) as pool_prod,
            tc.tile_pool(name="outs", bufs=4) as pool_out,
            tc.tile_pool(name="small", bufs=4) as pool_small,
            tc.tile_pool(name="persist", bufs=1) as pool_persist,
            tc.tile_pool(name="psum", bufs=1, space="PSUM") as pool_psum,
        ):
            # Persistent injection grids, alternating per chunk parity.
            # Zeroed once; only their boundary column (tdc/ac: never) is
            # rewritten afterwards, so the zero interior persists.
            fd1 = [pool_persist.tile([P, C * G], f32, name=f"fd1_{i}", tag=f"fd1_{i}") for i in range(2)]
            tdc = [pool_persist.tile([P, C * G], f32, name=f"tdc_{i}", tag=f"tdc_{i}") for i in range(2)]
            ac = [pool_persist.tile([P, C * G], f32, name=f"ac_{i}", tag=f"ac_{i}") for i in range(2)]
            for t in (*fd1, *tdc, *ac):
                nc.vector.memset(t[:], 0.0)

            def co3(t):  # c-order view of an l-major [P, LB*C] tile -> [P, C, LB]
                return t.rearrange("p (l c) -> p l c", c=C).transpose([0, 2, 1])

            def lm3(t):  # l-major view [P, LB, C], contiguous
                return t.rearrange("p (l c) -> p l c", c=C)

            def chunk_prologue(ch):
                r0 = ch * P
                st = {"ch": ch, "r0": r0}
                st["fd1_3"] = fd1[ch % 2].rearrange("p (c g) -> p c g", g=G)
                st["tdc_g"], st["ac_g"], st["fd1_g"] = \
                    tdc[ch % 2], ac[ch % 2], fd1[ch % 2]
                st["tdc_3"] = tdc[ch % 2].rearrange("p (c g) -> p c g", g=G)
                st["ac_3"] = ac[ch % 2].rearrange("p (c g) -> p c g", g=G)

                fd0 = pool_small.tile([P, C], f32, tag="fd0")
                ff0 = pool_small.tile([P, C], f32, tag="ff0")
                rmd = pool_small.tile([P, C], f32, tag="rmd")
                nc.sync.dma_start(out=fd0[:], in_=d_fx["flux_down_above_direct"][r0:r0 + P])
                nc.sync.dma_start(out=ff0[:], in_=d_fx["flux_down_above_diffuse"][r0:r0 + P])
                nc.sync.dma_start(out=rmd[:], in_=d_fx["r_multi_direct"][r0:r0 + P])
                st["fd0"], st["ff0"] = fd0, ff0

                # packed output tiles: [fds | ffs] and [ups | abs'] in
                # 64-col halves so each block needs only ONE reduce per tile
                # (abs is stored shifted +1 col; the epilogue re-slices)
                st["t_fdffs"] = pool_out.tile([P, 128], f32, name="t_fdffs",
                                              tag="o_fdffs")
                st["t_upabs"] = pool_out.tile([P, 128], f32, name="t_upabs",
                                              tag="o_upabs")

                # up_0 = sum_c fd0 * rmd  (accum_out of a fused mult)
                trash = pool_small.tile([P, C], f32, tag="trash")
                nc.vector.scalar_tensor_tensor(
                    out=trash[:], in0=fd0[:], scalar=1.0, in1=rmd[:],
                    op0=AL.mult, op1=AL.mult, accum_out=st["t_upabs"][:, 0:1])
                # level-0 channel sums of the initial fluxes
                nc.vector.tensor_reduce(out=st["t_fdffs"][:, 0:1], in_=fd0[:],
                                        axis=AX.X, op=AL.add)
                nc.vector.tensor_reduce(out=st["t_fdffs"][:, 64:65], in_=ff0[:],
                                        axis=AX.X, op=AL.add)
                return st

            lane3 = lambda t: t.rearrange("p (l c) -> p l c", c=C)

            def block_h1(st, bl):
                """First half of a block: loads, grid staging, fd scan,
                b-mult, fd-based products."""
                r0, l0 = st["r0"], bl * LB
                fd1_3, tdc_3, ac_3 = st["fd1_3"], st["tdc_3"], st["ac_3"]
                tin = {}
                for name in ("t_direct", "t_multi_direct"):
                    t = pool_in.tile([P, LB * C], f32, name=f"in_{name}",
                                     tag=f"in_{name}", bufs=3)
                    nc.sync.dma_start(
                        out=t[:],
                        in_=d_in[name][r0:r0 + P, l0:l0 + LB].rearrange(
                            "p l c -> p (l c)"))
                    tin[name] = t
                # all four product coefficients packed in one tile so the
                # whole product stage is a single DVE op
                coef = pool_in.tile([P, 4 * LB * C], f32, name="in_coef",
                                    tag="in_coef", bufs=2)
                for q, name in enumerate(("r_bottom_multi_direct",
                                          "r_bottom_multi_diffuse",
                                          "a_top_multi_direct",
                                          "a_top_multi_diffuse")):
                    nc.sync.dma_start(
                        out=coef[:, q * LB * C:(q + 1) * LB * C],
                        in_=d_in[name][r0:r0 + P, l0:l0 + LB].rearrange(
                            "p l c -> p (l c)"))

                # a = tdf + tmf computed by the DMA engines: load tdf, then
                # accumulate tmf onto it in the SDMA CCE (SWDGE path; the Q7
                # descriptor-generation cost is free now that GPSIMD idles)
                a_lm = pool_grid.tile([P, LB * C], f32, tag="a_lm", bufs=3)
                nc.sync.dma_start(
                    out=a_lm[:],
                    in_=d_in["t_diffuse"][r0:r0 + P, l0:l0 + LB].rearrange(
                        "p l c -> p (l c)"))
                nc.gpsimd.dma_start(
                    out=a_lm[:],
                    in_=d_in["t_multi_diffuse"][r0:r0 + P, l0:l0 + LB].rearrange(
                        "p l c -> p (l c)"),
                    accum_op=AL.add)

                # Input-only ACT copies (depend only on DMAs)
                nc.scalar.copy(out=tdc_3[:, :, 1:G], in_=co3(tin["t_direct"]))
                tmd_c = pool_grid.tile([P, C * G], f32, tag="tmd_c")
                tmdc_3 = tmd_c.rearrange("p (c g) -> p c g", g=G)
                nc.scalar.copy(out=tmdc_3[:, :, 1:G],
                               in_=co3(tin["t_multi_direct"]))
                nc.scalar.copy(out=ac_3[:, :, 1:G], in_=co3(a_lm))

                # carry-critical boundary copy on the (idle) GPSIMD
                if bl == 0:
                    nc.gpsimd.tensor_copy(out=fd1_3[:, :, 0:1],
                                          in_=st["fd0"][:].unsqueeze(2))
                else:
                    nc.gpsimd.tensor_copy(out=fd1_3[:, :, 0:1],
                                          in_=st["prev_fd_c3"][:, :, LB:LB + 1])

                fd_c = pool_seq.tile([P, C * G], f32, tag="fd_c")
                nc.vector.tensor_tensor_scan(
                    out=fd_c[:], data0=st["tdc_g"][:], data1=st["fd1_g"][:],
                    initial=0.0, op0=AL.mult, op1=AL.add)
                fd_c3 = fd_c.rearrange("p (c g) -> p c g", g=G)

                # b grid: col0 = ff carry, cols 1.. = fd_{l-1} * tmd_l
                b_buf = pool_grid.tile([P, C * G], f32, tag="b_buf", bufs=2)
                b_3 = b_buf.rearrange("p (c g) -> p c g", g=G)
                if bl == 0:
                    nc.gpsimd.tensor_copy(out=b_3[:, :, 0:1],
                                          in_=st["ff0"][:].unsqueeze(2))
                else:
                    nc.gpsimd.tensor_copy(out=b_3[:, :, 0:1],
                                          in_=st["prev_ff_c3"][:, :, LB:LB + 1])
                nc.gpsimd.tensor_mul(out=b_3[:, :, 1:G],
                                     in0=fd_c3[:, :, 0:LB],
                                     in1=tmdc_3[:, :, 1:G])
                lanes = pool_prod.tile([P, 2 * G * C], f32, tag="lanes", bufs=2)
                fd_lane = lanes[:, 0:G * C]

                # PSUM-resident: keeps ~30 KB/block of product write+read
                # traffic off the DMA-contended SBUF ports (PE is idle)
                quad = pool_psum.tile([P, 4 * LB * C], f32, tag="quad")
                return {"bl": bl, "tin": tin, "fd_c3": fd_c3, "b_buf": b_buf,
                        "b_3": b_3, "lanes": lanes, "fd_lane": fd_lane,
                        "coef": coef, "quad": quad}

            def block_h2(st, bx):
                """Second half: ff scan and ff-based products."""
                bl = bx["bl"]
                ff_c = pool_seq.tile([P, C * G], f32, tag="ff_c")
                nc.vector.tensor_tensor_scan(
                    out=ff_c[:], data0=st["ac_g"][:], data1=bx["b_buf"][:],
                    initial=0.0, op0=AL.mult, op1=AL.add)
                ff_c3 = ff_c.rearrange("p (c g) -> p c g", g=G)

                nc.scalar.copy(out=lane3(bx["fd_lane"]),
                               in_=bx["fd_c3"].transpose([0, 2, 1]))
                ff_lane = bx["lanes"][:, G * C:]
                nc.scalar.copy(out=lane3(ff_lane),
                               in_=ff_c3.transpose([0, 2, 1]))

                # single product op: lanes broadcast (step-0 dim) against
                # the packed coefficients -> quad = u0|u1|a0|a1
                lanes_b = bx["lanes"].rearrange("p (t x) -> p t x", t=2
                                                )[:, :, 0:LB * C].unsqueeze(1
                                                ).broadcast_to(
                                                    [P, 2, 2, LB * C])
                nc.vector.tensor_mul(
                    out=bx["quad"].rearrange("p (r t x) -> p r t x", r=2, t=2),
                    in0=lanes_b,
                    in1=bx["coef"].rearrange("p (r t x) -> p r t x", r=2, t=2))

                bx["ff_lane"] = ff_lane
                st["prev_fd_c3"], st["prev_ff_c3"] = bx["fd_c3"], ff_c3

            def block_tail(st, bx):
                """Block tail: the four DVE reductions (optionally emitted
                SKEW steps later in the stream)."""
                l0 = bx["bl"] * LB
                # one XY-reduce for up+ab: [P, pair, l, t, c], sum (t, c)
                q_red = bx["quad"].rearrange(
                    "p (r t l c) -> p r t l c", r=2, t=2, c=C
                ).transpose([0, 1, 3, 2, 4])
                nc.vector.tensor_reduce(
                    out=st["t_upabs"].rearrange("p (r x) -> p r x", r=2
                                                )[:, :, l0 + 1:l0 + LB + 1],
                    in_=q_red, axis=AX.XY, op=AL.add)
                # one X-reduce for both level-sum lanes (fd | ff)
                l_red = bx["lanes"].rearrange(
                    "p (t g c) -> p t g c", t=2, c=C)[:, :, 1:G, :]
                nc.vector.tensor_reduce(
                    out=st["t_fdffs"].rearrange("p (t x) -> p t x", t=2
                                                )[:, :, l0 + 1:l0 + LB + 1],
                    in_=l_red, axis=AX.X, op=AL.add)

            def chunk_epilogue(st):
                r0 = st["r0"]
                nc.sync.dma_start(out=d_fds[r0:r0 + P],
                                  in_=st["t_fdffs"][:, 0:L + 1])
                nc.sync.dma_start(out=d_ffs[r0:r0 + P],
                                  in_=st["t_fdffs"][:, 64:64 + L + 1])
                nc.sync.dma_start(out=d_ups[r0:r0 + P],
                                  in_=st["t_upabs"][:, 0:L + 1])
                nc.sync.dma_start(out=d_abs[r0:r0 + P],
                                  in_=st["t_upabs"][:, 65:65 + L])

            # Software-pipeline two independent chunks at block granularity
            # in one continuous stream: each engine's FIFO alternates
            # between two carry chains (no head-of-line stalls), and
            # prologues are emitted a few steps early so the next pair's
            # flux DMAs / output tiles are ready before its first block.
            seq = []
            for p in range(0, n_chunks, 2):
                for bl in range(NBLK):
                    for s_ in range(min(2, n_chunks - p)):
                        seq.append((p + s_, bl))
            states = {}

            def ensure_prologue(ch):
                if ch not in states:
                    states[ch] = chunk_prologue(ch)

            SKEW = 0
            pending = []  # (state, block-context) awaiting the skewed tail
            for idx, (ch, bl) in enumerate(seq):
                ensure_prologue(ch)
                for ch2, _ in seq[idx + 1:idx + 5]:
                    ensure_prologue(ch2)
                bx = block_h1(states[ch], bl)
                block_h2(states[ch], bx)
                pending.append((states[ch], bx))
                if len(pending) > SKEW:
                    st2, bx2 = pending.pop(0)
                    block_tail(st2, bx2)
                    if bx2["bl"] == NBLK - 1:
                        chunk_epilogue(st2)
            for st2, bx2 in pending:
                block_tail(st2, bx2)
                if bx2["bl"] == NBLK - 1:
                    chunk_epilogue(st2)

    nc.compile()
    return nc


_NC_CACHE = {}


def _get_nc(key=("full",)):
    if key not in _NC_CACHE:
        _NC_CACHE[key] = build_nc()
    return _NC_CACHE[key]


def kernel(**inputs):
    """Full-problem entry point: shard over 8 cores, run, gather."""
    from concourse.bass_utils import run_bass_kernel_spmd

    nc = _get_nc()
    rows = _B // _NCORES
    in_maps = []
    for core in range(_NCORES):
        sl = slice(core * rows, (core + 1) * rows)
        m = {n: np.ascontiguousarray(np.asarray(inputs[n])[sl], dtype=np.float32)
             for n in PROPS + FLUX}
        in_maps.append(m)

    res = run_bass_kernel_spmd(nc, in_maps, core_ids=list(range(_NCORES)))
    fds = np.concatenate([r["out_fds"] for r in res.results], axis=0)
    ffs = np.concatenate([r["out_ffs"] for r in res.results], axis=0)
    ups = np.concatenate([r["out_ups"] for r in res.results], axis=0)
    abs_ = np.concatenate([r["out_abs"] for r in res.results], axis=0)
    return fds, ffs, ups, abs_
